# revision 28
# baseline (speedup 1.0000x reference)
"""Trainium2 Bass kernel for nn_MoE (B=4,S=2048,D=1024,E=8,H=4D,top-2).

Expert-parallel across 8 NeuronCores: core e owns expert e's weights.

Pipeline per core:
  1. Gating (fp32) on its own 1/8 token shard, for all experts; top-2
     softmax coefficients computed with vector ops.
  2. AllToAll redistributes coefficient columns: core e receives
     coeff[:, e] for all 8192 tokens.
  3. Sparse path: tokens with coeff>0 are compacted (prefix-sum via
     triangular matmuls + indirect-DMA scatter of an index list),
     their rows gathered, FFN'd (fp32r matmuls), scaled by coeff and
     scattered into a zeroed [T, D] partial buffer.
  4. ReduceScatter sums partials across cores; each core emits its
     1/8 output shard; host concatenates.

kernel(**inputs) takes the full unsharded inputs and returns the full
[B, S, D] output. Self-contained: numpy + concourse only.
"""

import numpy as np

# Problem dims (hardcoded per spec)
B, S, D, E = 4, 2048, 1024, 8
H = 4 * D
T = B * S           # 8192 tokens
NC = 8              # cores
P = 128
TOPK = 2
NCAP = 2304         # compact-token capacity per expert (mean 2048, +6 sigma)


def build_moe(dims=None, dense=False, act="gelu", dbg=False,
              wdtype="f32r", ybf16=False):
    """Build the Bass module. Returns (nc, meta dict)."""
    import concourse.bacc as bacc
    import concourse.mybir as mybir
    import concourse.tile as tile
    from concourse.masks import make_identity, make_upper_triangular
    from concourse.bass import IndirectOffsetOnAxis

    dt = mybir.dt
    d_ = dims or {}
    Dd = d_.get("D", D)
    Hd = d_.get("H", H)
    Td = d_.get("T", T)
    Ed = d_.get("E", E)
    CAP = d_.get("NCAP", NCAP) if not dense else Td
    TPC = Td // NC          # tokens per core (gating shard / output shard)
    KD = Dd // P            # D k-tiles
    MH = Hd // P            # H m-tiles
    TB = 512                # max token block (psum/moving-operand limit)
    BLOCKS = []
    _o = 0
    while _o < CAP:
        _tb = min(TB, CAP - _o)
        BLOCKS.append((_o, _tb))
        _o += _tb
    NBLK = len(BLOCKS)
    DCH = 512 if Dd % 512 == 0 else Dd   # D output chunk
    ND = Dd // DCH
    NCOL = Td // P          # token columns in [P, NCOL] layouts
    assert TPC % P == 0 and CAP % P == 0 and Dd % DCH == 0 and DCH <= 512
    assert NCOL <= P

    f32 = dt.float32
    f32r = dt.float32r
    i32 = dt.int32
    wdt = {"f32r": dt.float32r, "bf16": dt.bfloat16}[wdtype]
    ydt = dt.bfloat16 if ybf16 else dt.float32
    AF = mybir.ActivationFunctionType
    ACTF = {"gelu": AF.Gelu, "tanh": AF.Tanh}[act]
    OP = mybir.AluOpType
    X = mybir.AxisListType.X
    SENT = 4 * Td  # sentinel index for padded slots (way out of range)

    nc = bacc.Bacc("TRN2", target_bir_lowering=False, debug=False,
                   num_devices=NC)

    # ---- I/O -------------------------------------------------------------
    xsT = nc.dram_tensor("xsT", [Dd, TPC], f32, kind="ExternalInput").ap()
    if dense:
        xT = nc.dram_tensor("xT", [Dd, Td], wdt, kind="ExternalInput").ap()
    else:
        xr = nc.dram_tensor("xr", [Td, Dd], f32, kind="ExternalInput").ap()
    w1 = nc.dram_tensor("w1", [Dd, Hd], wdt, kind="ExternalInput").ap()
    b1 = nc.dram_tensor("b1", [Hd], f32, kind="ExternalInput").ap()
    w2 = nc.dram_tensor("w2", [Hd, Dd], wdt, kind="ExternalInput").ap()
    b2 = nc.dram_tensor("b2", [Dd], wdt, kind="ExternalInput").ap()
    gw = nc.dram_tensor("gw", [Dd, Ed], f32, kind="ExternalInput").ap()
    gb = nc.dram_tensor("gb", [Ed], f32, kind="ExternalInput").ap()
    if not dense:
        iota16 = nc.dram_tensor("iota16", [16, Td // 16], f32,
                                kind="ExternalInput").ap()
        posj_in = nc.dram_tensor("posj", [P, CAP // P], f32,
                                 kind="ExternalInput").ap()
    out = nc.dram_tensor("out", [TPC, Dd], ydt, kind="ExternalOutput").ap()
    if dbg:
        _CAPd = d_.get("NCAP", NCAP) if not dense else Td
        dbg_idx = nc.dram_tensor("dbg_idx", [P, _CAPd // P], i32,
                                 kind="ExternalOutput").ap()
        dbg_ccomp = nc.dram_tensor("dbg_ccomp", [P, _CAPd // P], f32,
                                   kind="ExternalOutput").ap()
        dbg_nf = nc.dram_tensor("dbg_nf", [P, 1], f32,
                                kind="ExternalOutput").ap()
        _NT = TPC // P
        dbg_gall = nc.dram_tensor("dbg_gall", [P, _NT * Ed], f32,
                                  kind="ExternalOutput").ap()
        dbg_cfa = nc.dram_tensor("dbg_cfa", [P, _NT * Ed], f32,
                                 kind="ExternalOutput").ap()

    RG = [list(range(NC))]

    with tile.TileContext(nc) as tc:
        with (tc.tile_pool(name="dram", bufs=1, space="DRAM") as dram,
              tc.tile_pool(name="w2r", bufs=1) as w2rp):
            w2all = w2rp.tile([P, MH * Dd], wdt)
            zt = w2rp.tile([P, Dd], ydt)
            wz = w2rp.tile([NC, 16], f32)
            # tiny warm-up collectives: absorb ncfw init off the critical path
            wu_in = dram.tile([NC, 16], f32)
            wu_out = dram.tile([NC, 16], f32)
            wu_rs = dram.tile([16], f32)
            nc.vector.memset(wz[:], 0.0)
            nc.gpsimd.dma_start(out=wu_in[:, :], in_=wz[:])
            nc.gpsimd.collective_compute(
                "AllToAll", OP.bypass, replica_groups=RG,
                ins=[wu_in.opt()], outs=[wu_out.opt()])
            a2a_in = dram.tile([NC, TPC], f32)
            a2a_out = dram.tile([NC, TPC], f32)
            ybuf = dram.tile([Td, Dd], ydt)
            yshard = dram.tile([TPC, Dd], ydt)
            if not dense:
                idxfbuf = dram.tile([CAP], f32)
                cffbuf = dram.tile([CAP], f32)

                # zero the partial-output buffer early (overlaps gating);
                # scalar queue so it does not block gating loads or gathers
                nc.vector.memset(zt[:], 0.0)
                for i in range(Td // P):
                    nc.scalar.dma_start(
                        out=ybuf[i * P:(i + 1) * P, :], in_=zt[:])

            # ---- gating (own shard, all experts) -------------------------
            NT = TPC // P           # token tiles in shard
            with (tc.tile_pool(name="gat", bufs=1) as gp,
                  tc.tile_pool(name="gps", bufs=2, space="PSUM") as psg):
                gw_sb = gp.tile([P, KD * Ed], f32)
                nc.sync.dma_start(
                    out=gw_sb[:].rearrange("p (k e) -> p k e", k=KD),
                    in_=gw.rearrange("(k p) e -> p k e", p=P))
                gb_sb = gp.tile([1, Ed], f32)
                nc.sync.dma_start(out=gb_sb[:], in_=gb[None, :])
                ones1 = gp.tile([1, P], f32)
                nc.vector.memset(ones1[:], 1.0)
                ident8 = gp.tile([8, 8], f32)
                make_identity(nc, ident8[:])
                xsk = []
                for k in range(KD):
                    xk = gp.tile([P, TPC], f32, name=f"xsk{k}", tag=f"xsk{k}")
                    nc.sync.dma_start(out=xk[:],
                                      in_=xsT[k * P:(k + 1) * P, :])
                    xsk.append(xk)
                # W2 resident load (no deps; scalar queue is idle early)
                for hk in range(MH):
                    nc.scalar.dma_start(
                        out=w2all[:, hk * Dd:(hk + 1) * Dd],
                        in_=w2[hk * P:(hk + 1) * P, :])

                # gatesT [E, tok]: stationary gw chunks, moving x
                gts = gp.tile([8, TPC], f32)
                GTB = min(TB, TPC)
                for sl in range(TPC // GTB):
                    pgt = psg.tile([8, GTB], f32, tag="pgt")
                    for k in range(KD):
                        nc.tensor.matmul(
                            pgt[:Ed, :], lhsT=gw_sb[:, k * Ed:(k + 1) * Ed],
                            rhs=xsk[k][:, sl * GTB:(sl + 1) * GTB],
                            start=(k == 0), stop=(k == KD - 1))
                    nc.vector.tensor_copy(gts[:Ed, sl * GTB:(sl + 1) * GTB],
                                          pgt[:Ed, :])
                # transpose to [tok, E] tiles, add gate bias via rank-1
                gall = gp.tile([P, NT * Ed], f32)
                for mt in range(NT):
                    pg = psg.tile([P, Ed], f32, tag="pg")
                    nc.tensor.matmul(pg[:, :Ed],
                                     lhsT=gts[:Ed, mt * P:(mt + 1) * P],
                                     rhs=ident8[:], is_transpose=True,
                                     start=True, stop=False)
                    nc.tensor.matmul(pg[:, :Ed], lhsT=ones1[:], rhs=gb_sb[:],
                                     start=False, stop=True)
                    nc.vector.tensor_copy(gall[:, mt * Ed:(mt + 1) * Ed],
                                          pg[:, :Ed])
                # batched top-2 softmax coefficients over all NT tiles
                g3 = gall[:].rearrange("p (t e) -> p t e", e=Ed)
                m1a = gp.tile([P, NT], f32)
                nc.vector.reduce_max(m1a[:], g3, axis=X)
                m1b = m1a[:].unsqueeze(2).to_broadcast([P, NT, Ed])
                gmx = gp.tile([P, NT * Ed], f32)
                g3mx = gmx[:].rearrange("p (t e) -> p t e", e=Ed)
                nc.vector.tensor_tensor(g3mx, g3, m1b, op=OP.subtract)
                exa = gp.tile([P, NT * Ed], f32)
                nc.scalar.activation(exa[:], gmx[:], AF.Exp)
                eqa = gp.tile([P, NT * Ed], f32)
                nc.vector.tensor_tensor(
                    eqa[:].rearrange("p (t e) -> p t e", e=Ed),
                    g3, m1b, op=OP.is_equal)
                nc.vector.tensor_scalar(eqa[:], eqa[:], -1e30, None,
                                        op0=OP.mult)
                nc.vector.tensor_add(eqa[:], eqa[:], gall[:])
                m2a = gp.tile([P, NT], f32)
                nc.vector.reduce_max(
                    m2a[:], eqa[:].rearrange("p (t e) -> p t e", e=Ed),
                    axis=X)
                m2b = m2a[:].unsqueeze(2).to_broadcast([P, NT, Ed])
                sela = gp.tile([P, NT * Ed], f32)
                nc.vector.tensor_tensor(
                    sela[:].rearrange("p (t e) -> p t e", e=Ed),
                    g3, m2b, op=OP.is_ge)
                dm = gp.tile([P, NT], f32)
                nc.vector.tensor_sub(dm[:], m2a[:], m1a[:])
                nc.scalar.activation(dm[:], dm[:], AF.Exp)
                nc.vector.tensor_scalar_add(dm[:], dm[:], 1.0)
                nc.vector.reciprocal(dm[:], dm[:])
                cfa = gp.tile([P, NT * Ed], f32)
                nc.vector.tensor_mul(cfa[:], sela[:], exa[:])
                dmb = dm[:].unsqueeze(2).to_broadcast([P, NT, Ed])
                nc.vector.tensor_tensor(
                    cfa[:].rearrange("p (t e) -> p t e", e=Ed),
                    cfa[:].rearrange("p (t e) -> p t e", e=Ed),
                    dmb, op=OP.mult)
                for j in range(NC):
                    nc.sync.dma_start(
                        out=a2a_in[j:j + 1, :].rearrange("o (t p) -> (o p) t",
                                                         p=P),
                        in_=cfa[:].rearrange("p (t e) -> p t e",
                                             e=Ed)[:, :, j])
                if dbg:
                    nc.sync.dma_start(out=dbg_gall, in_=gall[:])
                    nc.sync.dma_start(out=dbg_cfa, in_=cfa[:])

            nc.gpsimd.collective_compute(
                "AllToAll", OP.bypass, replica_groups=RG,
                ins=[a2a_in.opt()], outs=[a2a_out.opt()])

            # ---- constants + coeff column -------------------------------
            with tc.tile_pool(name="cst", bufs=1) as cst:
                if dense:
                    ccol = cst.tile([P, NCOL], f32)
                    nc.sync.dma_start(
                        out=ccol[:],
                        in_=a2a_out[:].rearrange("r (c p) -> p (r c)", p=P))
                b1s = cst.tile([P, MH], f32)
                nc.sync.dma_start(out=b1s[:],
                                  in_=b1.rearrange("(m p) -> p m", p=P))
                b2s = cst.tile([1, Dd], wdt)
                nc.sync.dma_start(out=b2s[:], in_=b2[None, :])
                ones1f = cst.tile([1, P], f32)
                nc.vector.memset(ones1f[:], 1.0)
                ones1r = cst.tile([1, P], wdt)
                nc.vector.tensor_copy(ones1r[:], ones1f[:])

                if not dense:
                    # ---- compaction via gpsimd sparse_gather ------------
                    F16 = Td // 16
                    C16 = CAP // 16
                    with (tc.tile_pool(name="cmp", bufs=1) as cp,
                          tc.tile_pool(name="cps", bufs=1, space="PSUM") as cps):
                        cc16 = cp.tile([16, F16], f32)
                        nc.sync.dma_start(
                            out=cc16[:],
                            in_=a2a_out[:]
                                .rearrange("r (f p) -> (r f) p", p=16)
                                .rearrange("g p -> p g"))
                        io16 = cp.tile([16, F16], f32)
                        nc.sync.dma_start(out=io16[:], in_=iota16)
                        m16 = cp.tile([16, F16], f32)
                        nc.vector.tensor_scalar(m16[:], cc16[:], 0.0, None,
                                                op0=OP.is_gt)
                        cand_i = cp.tile([16, F16], f32)
                        nc.vector.tensor_mul(cand_i[:], m16[:], io16[:])
                        nc.vector.tensor_scalar_add(cand_i[:], cand_i[:], -1.0)
                        cand_c = cp.tile([16, F16], f32)
                        nc.vector.tensor_scalar_add(cand_c[:], cc16[:], 1.0)
                        nc.vector.tensor_mul(cand_c[:], m16[:], cand_c[:])
                        nc.vector.tensor_scalar_add(cand_c[:], cand_c[:], -1.0)
                        sg_i = cp.tile([16, C16], f32)
                        nf = cp.tile([1, 1], dt.uint32)
                        nc.gpsimd.sparse_gather(sg_i[:], cand_i[:],
                                                num_found=nf[:])
                        sg_c = cp.tile([16, C16], f32)
                        nf2 = cp.tile([1, 1], dt.uint32)
                        nc.gpsimd.sparse_gather(sg_c[:], cand_c[:],
                                                num_found=nf2[:])
                        nc.sync.dma_start(
                            out=idxfbuf.rearrange("(f p) -> p f", p=16),
                            in_=sg_i[:])
                        nc.sync.dma_start(
                            out=cffbuf.rearrange("(f p) -> p f", p=16),
                            in_=sg_c[:])
                        # broadcast num_found to all partitions via rank-1 mm
                        nf_f = cp.tile([1, 1], f32)
                        nc.vector.tensor_copy(nf_f[:], nf[:])
                        nf_ps = cps.tile([P, 1], f32)
                        nc.tensor.matmul(nf_ps[:], lhsT=ones1f[:],
                                         rhs=nf_f[:], start=True, stop=True)
                        nf_bcast = cst.tile([P, 1], f32)
                        nc.vector.tensor_copy(nf_bcast[:], nf_ps[:])

                # ---- FFN ----------------------------------------------
                with (tc.tile_pool(name="idx", bufs=1) as ip,
                      tc.tile_pool(name="xtp", bufs=2) as xtp,
                      tc.tile_pool(name="wp", bufs=3) as wp,
                      tc.tile_pool(name="hp", bufs=1) as hp,
                      tc.tile_pool(name="yp", bufs=2) as yp,
                      tc.tile_pool(name="ps1", bufs=2, space="PSUM") as ps1,
                      tc.tile_pool(name="ps2", bufs=1, space="PSUM") as ps2):
                    if not dense:
                        NBC = CAP // P
                        idxf = ip.tile([P, NBC], f32)
                        nc.sync.dma_start(
                            out=idxf[:],
                            in_=idxfbuf.rearrange("(c p) -> p c", p=P))
                        cf_sb = ip.tile([P, NBC], f32)
                        nc.sync.dma_start(
                            out=cf_sb[:],
                            in_=cffbuf.rearrange("(c p) -> p c", p=P))
                        posj = ip.tile([P, NBC], f32)
                        nc.sync.dma_start(out=posj[:], in_=posj_in)
                        nf_bc = nf_bcast
                        inval = ip.tile([P, NBC], i32)
                        nc.vector.tensor_scalar(inval[:], posj[:],
                                                nf_bc[:, 0:1], None,
                                                op0=OP.is_ge)
                        sntf = ip.tile([P, NBC], f32)
                        nc.vector.memset(sntf[:], float(SENT))
                        idxe = ip.tile([P, NBC], f32)
                        nc.vector.select(idxe[:], inval[:], sntf[:], idxf[:])
                        idx_sb = ip.tile([P, NBC], i32)
                        nc.vector.tensor_copy(idx_sb[:], idxe[:])
                        gidx = ip.tile([P, NBC], i32)
                        nc.vector.tensor_scalar(gidx[:], idx_sb[:], Td - 1,
                                                0, op0=OP.min, op1=OP.max)
                        ident = ip.tile([P, P], f32)
                        make_identity(nc, ident[:])
                        if dbg:
                            nc.sync.dma_start(out=dbg_idx, in_=idx_sb[:])
                            nc.sync.dma_start(out=dbg_ccomp, in_=cf_sb[:])
                            nc.sync.dma_start(out=dbg_nf, in_=nf_bc[:])
                    for blk in range(NBLK):
                        ts0, tb = BLOCKS[blk]
                        mt_n = tb // P
                        xts = []
                        if dense:
                            for k in range(KD):
                                xt = xtp.tile([P, tb], wdt, tag=f"xt{k}",
                                              name=f"xt{k}")
                                nc.sync.dma_start(
                                    out=xt[:],
                                    in_=xT[k * P:(k + 1) * P, ts0:ts0 + tb])
                                xts.append(xt)
                        else:
                            for k in range(KD):
                                xt = xtp.tile([P, tb], wdt, tag=f"xt{k}",
                                              name=f"xt{k}")
                                xts.append(xt)
                            for j in range(mt_n):
                                c = ts0 // P + j
                                xg = xtp.tile([P, Dd], f32, tag="xg")
                                nc.gpsimd.indirect_dma_start(
                                    out=xg[:], out_offset=None,
                                    in_=xr,
                                    in_offset=IndirectOffsetOnAxis(
                                        ap=gidx[:, c:c + 1], axis=0))
                                for k in range(KD):
                                    pt = ps1.tile([P, P], f32, tag="ptr")
                                    nc.tensor.transpose(
                                        pt[:], xg[:, k * P:(k + 1) * P],
                                        ident[:])
                                    nc.vector.tensor_copy(
                                        xts[k][:, j * P:(j + 1) * P], pt[:])
                        hts = []
                        for m in range(MH):
                            w1m = wp.tile([P, KD * P], wdt, tag="w1m")
                            nc.sync.dma_start(
                                out=w1m[:].rearrange("p (k h) -> p k h", k=KD),
                                in_=w1[:, m * P:(m + 1) * P]
                                    .rearrange("(k p) h -> p k h", p=P))
                            ph = ps1.tile([P, tb], f32, tag="ph")
                            for k in range(KD):
                                nc.tensor.matmul(
                                    ph[:], lhsT=w1m[:, k * P:(k + 1) * P],
                                    rhs=xts[k][:],
                                    start=(k == 0), stop=(k == KD - 1))
                            ht = hp.tile([P, tb], wdt, tag=f"ht{m}")
                            nc.scalar.activation(ht[:], ph[:], ACTF,
                                                 bias=b1s[:, m:m + 1],
                                                 scale=1.0)
                            hts.append(ht)
                        for d in range(ND):
                            pys = [ps2.tile([P, DCH], f32, tag=f"py{mt}",
                                            name=f"py{mt}")
                                   for mt in range(mt_n)]
                            for hk in range(MH):
                                for mt in range(mt_n):
                                    nc.tensor.matmul(
                                        pys[mt][:],
                                        lhsT=hts[hk][:, mt * P:(mt + 1) * P],
                                        rhs=w2all[:, hk * Dd + d * DCH:
                                                  hk * Dd + (d + 1) * DCH],
                                        start=(hk == 0), stop=False)
                            for mt in range(mt_n):
                                nc.tensor.matmul(
                                    pys[mt][:], lhsT=ones1r[:],
                                    rhs=b2s[:, d * DCH:(d + 1) * DCH],
                                    start=False, stop=True)
                            for mt in range(mt_n):
                                yq = yp.tile([P, DCH], ydt, tag=f"yq{mt}",
                                             name=f"yq{mt}")
                                c = ts0 // P + mt
                                if dense:
                                    nc.vector.tensor_scalar_mul(
                                        yq[:], pys[mt][:],
                                        ccol[:, c:c + 1])
                                    nc.sync.dma_start(
                                        out=ybuf[ts0 + mt * P:
                                                 ts0 + (mt + 1) * P,
                                                 d * DCH:(d + 1) * DCH],
                                        in_=yq[:])
                                else:
                                    nc.vector.tensor_scalar_mul(
                                        yq[:], pys[mt][:],
                                        cf_sb[:, c:c + 1])
                                    nc.gpsimd.indirect_dma_start(
                                        out=ybuf[:],
                                        out_offset=IndirectOffsetOnAxis(
                                            ap=idx_sb[:, c:c + 1], axis=0),
                                        in_=yq[:], in_offset=None,
                                        element_offset=d * DCH,
                                        bounds_check=Td - 1,
                                        oob_is_err=False)

            nc.gpsimd.collective_compute(
                "ReduceScatter", OP.add, replica_groups=RG,
                ins=[ybuf.opt()], outs=[yshard.opt()])
            nc.sync.dma_start(out=out, in_=yshard[:])

    nc.compile()
    meta = dict(D=Dd, H=Hd, T=Td, E=Ed, TPC=TPC, CAP=CAP)
    return nc, meta


# ----------------------------------------------------------------------------
def make_in_maps(inputs, dims=None, dense=False, wdtype="f32r"):
    """Shard full inputs into per-core input maps (host-side, numpy only)."""
    d_ = dims or {}
    Td = d_.get("T", T)
    Dd = d_.get("D", D)
    TPC = Td // NC
    x = np.asarray(inputs["x"], dtype=np.float32)
    x2 = np.ascontiguousarray(x.reshape(Td, Dd))
    temp = np.float32(inputs["temperature"])
    gws = np.ascontiguousarray(np.asarray(inputs["gate_w"], np.float32) / temp)
    gbs = np.ascontiguousarray(np.asarray(inputs["gate_b"], np.float32) / temp)
    W1 = np.asarray(inputs["W1"], np.float32)
    b1_ = np.asarray(inputs["b1"], np.float32)
    W2 = np.asarray(inputs["W2"], np.float32)
    b2_ = np.asarray(inputs["b2"], np.float32)
    if wdtype == "bf16":
        import ml_dtypes
        wnp = ml_dtypes.bfloat16
    else:
        wnp = np.float32
    W1 = W1.astype(wnp)
    W2 = W2.astype(wnp)
    b2_ = b2_.astype(wnp)
    if dense:
        xT_np = np.ascontiguousarray(x2.T).astype(wnp)
    else:
        CAP = d_.get("NCAP", NCAP)
        F16 = Td // 16
        iota16_np = ((np.arange(16)[:, None] + 16 * np.arange(F16)[None, :])
                     .astype(np.float32) + 1.0)
        posj_np = (np.arange(CAP // P)[None, :] * P
                   + np.arange(P)[:, None]).astype(np.float32)
    in_maps = []
    for rk in range(NC):
        m = {
            "xsT": np.ascontiguousarray(x2[rk * TPC:(rk + 1) * TPC].T),
            "w1": np.ascontiguousarray(W1[rk]),
            "b1": np.ascontiguousarray(b1_[rk]),
            "w2": np.ascontiguousarray(W2[rk]),
            "b2": np.ascontiguousarray(b2_[rk]),
            "gw": gws,
            "gb": gbs,
        }
        if dense:
            m["xT"] = xT_np
        else:
            m["xr"] = x2
            m["iota16"] = iota16_np
            m["posj"] = posj_np
        in_maps.append(m)
    return in_maps


_BUILT = {}


def run_hw(inputs, dims=None, trace=False, act="gelu", dense=False,
           wdtype="f32r", ybf16=False):
    """Run on hardware via run_bass_kernel_spmd; returns (out_full, results)."""
    from concourse.bass_utils import run_bass_kernel_spmd
    key = (dense, act, wdtype, ybf16, tuple(sorted((dims or {}).items())))
    if key not in _BUILT:
        _BUILT[key] = build_moe(dims=dims, dense=dense, act=act,
                                wdtype=wdtype, ybf16=ybf16)
    nc, meta = _BUILT[key]
    in_maps = make_in_maps(inputs, dims=dims, dense=dense, wdtype=wdtype)
    res = run_bass_kernel_spmd(nc, in_maps, list(range(NC)), trace=trace)
    shards = [np.asarray(res.results[i]["out"], dtype=np.float32)
              for i in range(NC)]
    out_full = np.concatenate(shards, axis=0)
    if not dims:
        out_full = out_full.reshape(B, S, D)
    return out_full, res


def kernel(**inputs):
    out, _ = run_hw(inputs, dims=None, trace=False, dense=False)
    return out


# revision 29
# speedup vs baseline: 1.0624x; 1.0624x over previous
"""Trainium2 Bass kernel for nn_MoE (B=4,S=2048,D=1024,E=8,H=4D,top-2).

Expert-parallel across 8 NeuronCores: core e owns expert e's weights.

Pipeline per core:
  1. Gating (fp32) on its own 1/8 token shard, for all experts; top-2
     softmax coefficients computed with vector ops.
  2. AllToAll redistributes coefficient columns: core e receives
     coeff[:, e] for all 8192 tokens.
  3. Sparse path: tokens with coeff>0 are compacted (prefix-sum via
     triangular matmuls + indirect-DMA scatter of an index list),
     their rows gathered, FFN'd (fp32r matmuls), scaled by coeff and
     scattered into a zeroed [T, D] partial buffer.
  4. ReduceScatter sums partials across cores; each core emits its
     1/8 output shard; host concatenates.

kernel(**inputs) takes the full unsharded inputs and returns the full
[B, S, D] output. Self-contained: numpy + concourse only.
"""

import numpy as np

# Problem dims (hardcoded per spec)
B, S, D, E = 4, 2048, 1024, 8
H = 4 * D
T = B * S           # 8192 tokens
NC = 8              # cores
P = 128
TOPK = 2
NCAP = 2304         # compact-token capacity per expert (mean 2048, +6 sigma)


def build_moe(dims=None, dense=False, act="gelu", dbg=False,
              wdtype="f32r", ybf16=False):
    """Build the Bass module. Returns (nc, meta dict)."""
    import concourse.bacc as bacc
    import concourse.mybir as mybir
    import concourse.tile as tile
    from concourse.masks import make_identity, make_upper_triangular
    from concourse.bass import IndirectOffsetOnAxis

    dt = mybir.dt
    d_ = dims or {}
    Dd = d_.get("D", D)
    Hd = d_.get("H", H)
    Td = d_.get("T", T)
    Ed = d_.get("E", E)
    CAP = d_.get("NCAP", NCAP) if not dense else Td
    TPC = Td // NC          # tokens per core (gating shard / output shard)
    KD = Dd // P            # D k-tiles
    MH = Hd // P            # H m-tiles
    TB = 512                # max token block (psum/moving-operand limit)
    BLOCKS = []
    _o = 0
    while _o < CAP:
        _tb = min(TB, CAP - _o)
        BLOCKS.append((_o, _tb))
        _o += _tb
    NBLK = len(BLOCKS)
    DCH = 512 if Dd % 512 == 0 else Dd   # D output chunk
    ND = Dd // DCH
    NCOL = Td // P          # token columns in [P, NCOL] layouts
    assert TPC % P == 0 and CAP % P == 0 and Dd % DCH == 0 and DCH <= 512
    assert NCOL <= P

    f32 = dt.float32
    f32r = dt.float32r
    i32 = dt.int32
    wdt = {"f32r": dt.float32r, "bf16": dt.bfloat16}[wdtype]
    ydt = dt.bfloat16 if ybf16 else dt.float32
    AF = mybir.ActivationFunctionType
    ACTF = {"gelu": AF.Gelu, "tanh": AF.Tanh}[act]
    OP = mybir.AluOpType
    X = mybir.AxisListType.X
    SENT = 4 * Td  # sentinel index for padded slots (way out of range)

    nc = bacc.Bacc("TRN2", target_bir_lowering=False, debug=False,
                   num_devices=NC)

    # ---- I/O -------------------------------------------------------------
    xsT = nc.dram_tensor("xsT", [Dd, TPC], f32, kind="ExternalInput").ap()
    if dense:
        xT = nc.dram_tensor("xT", [Dd, Td], wdt, kind="ExternalInput").ap()
    else:
        xr = nc.dram_tensor("xr", [Td, Dd], f32, kind="ExternalInput").ap()
    w1 = nc.dram_tensor("w1", [MH, P, KD * P], wdt,
                        kind="ExternalInput").ap()
    b1 = nc.dram_tensor("b1", [Hd], f32, kind="ExternalInput").ap()
    w2 = nc.dram_tensor("w2", [Hd, Dd], wdt, kind="ExternalInput").ap()
    b2 = nc.dram_tensor("b2", [Dd], wdt, kind="ExternalInput").ap()
    gw = nc.dram_tensor("gw", [Dd, Ed], f32, kind="ExternalInput").ap()
    gb = nc.dram_tensor("gb", [Ed], f32, kind="ExternalInput").ap()
    if not dense:
        iota16 = nc.dram_tensor("iota16", [16, Td // 16], f32,
                                kind="ExternalInput").ap()
        posj_in = nc.dram_tensor("posj", [P, CAP // P], f32,
                                 kind="ExternalInput").ap()
    out = nc.dram_tensor("out", [TPC, Dd], ydt, kind="ExternalOutput").ap()
    if dbg:
        _CAPd = d_.get("NCAP", NCAP) if not dense else Td
        dbg_idx = nc.dram_tensor("dbg_idx", [P, _CAPd // P], i32,
                                 kind="ExternalOutput").ap()
        dbg_ccomp = nc.dram_tensor("dbg_ccomp", [P, _CAPd // P], f32,
                                   kind="ExternalOutput").ap()
        dbg_nf = nc.dram_tensor("dbg_nf", [P, 1], f32,
                                kind="ExternalOutput").ap()
        _NT = TPC // P
        dbg_gall = nc.dram_tensor("dbg_gall", [P, _NT * Ed], f32,
                                  kind="ExternalOutput").ap()
        dbg_cfa = nc.dram_tensor("dbg_cfa", [P, _NT * Ed], f32,
                                 kind="ExternalOutput").ap()

    RG = [list(range(NC))]

    with tile.TileContext(nc) as tc:
        with (tc.tile_pool(name="dram", bufs=1, space="DRAM") as dram,
              tc.tile_pool(name="w2r", bufs=1) as w2rp):
            w2all = w2rp.tile([P, MH * Dd], wdt)
            zt = w2rp.tile([P, 8 * Dd], ydt)
            wz = w2rp.tile([NC, 16], f32)
            # tiny warm-up collectives: absorb ncfw init off the critical path
            wu_in = dram.tile([NC, 16], f32)
            wu_out = dram.tile([NC, 16], f32)
            wu_rs = dram.tile([16], f32)
            nc.vector.memset(wz[:], 0.0)
            nc.gpsimd.dma_start(out=wu_in[:, :], in_=wz[:])
            nc.gpsimd.collective_compute(
                "AllToAll", OP.bypass, replica_groups=RG,
                ins=[wu_in.opt()], outs=[wu_out.opt()])
            a2a_in = dram.tile([NC, TPC], f32)
            a2a_out = dram.tile([NC, TPC], f32)
            ybuf = dram.tile([Td, Dd], ydt)
            yshard = dram.tile([TPC, Dd], ydt)
            if not dense:
                idxfbuf = dram.tile([CAP], f32)
                cffbuf = dram.tile([CAP], f32)

                # zero the partial-output buffer early (overlaps gating);
                # scalar queue so it does not block gating loads or gathers
                nc.vector.memset(zt[:], 0.0)
                ZR = 8 * P
                for i in range(Td // ZR):
                    nc.scalar.dma_start(
                        out=ybuf[i * ZR:(i + 1) * ZR, :]
                            .rearrange("(a b) d -> a (b d)", a=P),
                        in_=zt[:])

            # ---- gating (own shard, all experts) -------------------------
            NT = TPC // P           # token tiles in shard
            with (tc.tile_pool(name="gat", bufs=1) as gp,
                  tc.tile_pool(name="gps", bufs=2, space="PSUM") as psg):
                gw_sb = gp.tile([P, KD * Ed], f32)
                nc.sync.dma_start(
                    out=gw_sb[:].rearrange("p (k e) -> p k e", k=KD),
                    in_=gw.rearrange("(k p) e -> p k e", p=P))
                gb_sb = gp.tile([1, Ed], f32)
                nc.sync.dma_start(out=gb_sb[:], in_=gb[None, :])
                ones1 = gp.tile([1, P], f32)
                nc.vector.memset(ones1[:], 1.0)
                ident8 = gp.tile([8, 8], f32)
                make_identity(nc, ident8[:])
                xsk = []
                for k in range(KD):
                    xk = gp.tile([P, TPC], f32, name=f"xsk{k}", tag=f"xsk{k}")
                    nc.sync.dma_start(out=xk[:],
                                      in_=xsT[k * P:(k + 1) * P, :])
                    xsk.append(xk)
                # W2 resident load (no deps; scalar queue is idle early)
                for hk in range(MH):
                    nc.scalar.dma_start(
                        out=w2all[:, hk * Dd:(hk + 1) * Dd],
                        in_=w2[hk * P:(hk + 1) * P, :])

                # gatesT [E, tok]: stationary gw chunks, moving x
                gts = gp.tile([8, TPC], f32)
                GTB = min(TB, TPC)
                for sl in range(TPC // GTB):
                    pgt = psg.tile([8, GTB], f32, tag="pgt")
                    for k in range(KD):
                        nc.tensor.matmul(
                            pgt[:Ed, :], lhsT=gw_sb[:, k * Ed:(k + 1) * Ed],
                            rhs=xsk[k][:, sl * GTB:(sl + 1) * GTB],
                            start=(k == 0), stop=(k == KD - 1))
                    nc.vector.tensor_copy(gts[:Ed, sl * GTB:(sl + 1) * GTB],
                                          pgt[:Ed, :])
                # transpose to [tok, E] tiles, add gate bias via rank-1
                gall = gp.tile([P, NT * Ed], f32)
                for mt in range(NT):
                    pg = psg.tile([P, Ed], f32, tag="pg")
                    nc.tensor.matmul(pg[:, :Ed],
                                     lhsT=gts[:Ed, mt * P:(mt + 1) * P],
                                     rhs=ident8[:], is_transpose=True,
                                     start=True, stop=False)
                    nc.tensor.matmul(pg[:, :Ed], lhsT=ones1[:], rhs=gb_sb[:],
                                     start=False, stop=True)
                    nc.vector.tensor_copy(gall[:, mt * Ed:(mt + 1) * Ed],
                                          pg[:, :Ed])
                # batched top-2 softmax coefficients over all NT tiles
                g3 = gall[:].rearrange("p (t e) -> p t e", e=Ed)
                m1a = gp.tile([P, NT], f32)
                nc.vector.reduce_max(m1a[:], g3, axis=X)
                m1b = m1a[:].unsqueeze(2).to_broadcast([P, NT, Ed])
                gmx = gp.tile([P, NT * Ed], f32)
                g3mx = gmx[:].rearrange("p (t e) -> p t e", e=Ed)
                nc.vector.tensor_tensor(g3mx, g3, m1b, op=OP.subtract)
                exa = gp.tile([P, NT * Ed], f32)
                nc.scalar.activation(exa[:], gmx[:], AF.Exp)
                eqa = gp.tile([P, NT * Ed], f32)
                nc.vector.tensor_tensor(
                    eqa[:].rearrange("p (t e) -> p t e", e=Ed),
                    g3, m1b, op=OP.is_equal)
                nc.vector.tensor_scalar(eqa[:], eqa[:], -1e30, None,
                                        op0=OP.mult)
                nc.vector.tensor_add(eqa[:], eqa[:], gall[:])
                m2a = gp.tile([P, NT], f32)
                nc.vector.reduce_max(
                    m2a[:], eqa[:].rearrange("p (t e) -> p t e", e=Ed),
                    axis=X)
                m2b = m2a[:].unsqueeze(2).to_broadcast([P, NT, Ed])
                sela = gp.tile([P, NT * Ed], f32)
                nc.vector.tensor_tensor(
                    sela[:].rearrange("p (t e) -> p t e", e=Ed),
                    g3, m2b, op=OP.is_ge)
                dm = gp.tile([P, NT], f32)
                nc.vector.tensor_sub(dm[:], m2a[:], m1a[:])
                nc.scalar.activation(dm[:], dm[:], AF.Exp)
                nc.vector.tensor_scalar_add(dm[:], dm[:], 1.0)
                nc.vector.reciprocal(dm[:], dm[:])
                cfa = gp.tile([P, NT * Ed], f32)
                nc.vector.tensor_mul(cfa[:], sela[:], exa[:])
                dmb = dm[:].unsqueeze(2).to_broadcast([P, NT, Ed])
                nc.vector.tensor_tensor(
                    cfa[:].rearrange("p (t e) -> p t e", e=Ed),
                    cfa[:].rearrange("p (t e) -> p t e", e=Ed),
                    dmb, op=OP.mult)
                for j in range(NC):
                    nc.gpsimd.dma_start(
                        out=a2a_in[j:j + 1, :].rearrange("o (t p) -> (o p) t",
                                                         p=P),
                        in_=cfa[:].rearrange("p (t e) -> p t e",
                                             e=Ed)[:, :, j])
                if dbg:
                    nc.sync.dma_start(out=dbg_gall, in_=gall[:])
                    nc.sync.dma_start(out=dbg_cfa, in_=cfa[:])

            nc.gpsimd.collective_compute(
                "AllToAll", OP.bypass, replica_groups=RG,
                ins=[a2a_in.opt()], outs=[a2a_out.opt()])

            # ---- constants + coeff column -------------------------------
            with tc.tile_pool(name="cst", bufs=1) as cst:
                if dense:
                    ccol = cst.tile([P, NCOL], f32)
                    nc.sync.dma_start(
                        out=ccol[:],
                        in_=a2a_out[:].rearrange("r (c p) -> p (r c)", p=P))
                b1s = cst.tile([P, MH], f32)
                nc.sync.dma_start(out=b1s[:],
                                  in_=b1.rearrange("(m p) -> p m", p=P))
                b2s = cst.tile([1, Dd], wdt)
                nc.sync.dma_start(out=b2s[:], in_=b2[None, :])
                ones1f = cst.tile([1, P], f32)
                nc.vector.memset(ones1f[:], 1.0)
                ones1r = cst.tile([1, P], wdt)
                nc.vector.tensor_copy(ones1r[:], ones1f[:])

                if not dense:
                    # ---- compaction via gpsimd sparse_gather ------------
                    F16 = Td // 16
                    C16 = CAP // 16
                    with (tc.tile_pool(name="cmp", bufs=1) as cp,
                          tc.tile_pool(name="cps", bufs=1, space="PSUM") as cps):
                        cc16 = cp.tile([16, F16], f32)
                        nc.sync.dma_start(
                            out=cc16[:],
                            in_=a2a_out[:]
                                .rearrange("r (f p) -> (r f) p", p=16)
                                .rearrange("g p -> p g"))
                        io16 = cp.tile([16, F16], f32)
                        nc.sync.dma_start(out=io16[:], in_=iota16)
                        m16 = cp.tile([16, F16], f32)
                        nc.vector.tensor_scalar(m16[:], cc16[:], 0.0, None,
                                                op0=OP.is_gt)
                        cand_i = cp.tile([16, F16], f32)
                        nc.vector.tensor_mul(cand_i[:], m16[:], io16[:])
                        nc.vector.tensor_scalar_add(cand_i[:], cand_i[:], -1.0)
                        cand_c = cp.tile([16, F16], f32)
                        nc.vector.tensor_scalar_add(cand_c[:], cc16[:], 1.0)
                        nc.vector.tensor_mul(cand_c[:], m16[:], cand_c[:])
                        nc.vector.tensor_scalar_add(cand_c[:], cand_c[:], -1.0)
                        sg_i = cp.tile([16, C16], f32)
                        nf = cp.tile([1, 1], dt.uint32)
                        nc.gpsimd.sparse_gather(sg_i[:], cand_i[:],
                                                num_found=nf[:])
                        sg_c = cp.tile([16, C16], f32)
                        nf2 = cp.tile([1, 1], dt.uint32)
                        nc.gpsimd.sparse_gather(sg_c[:], cand_c[:],
                                                num_found=nf2[:])
                        nc.sync.dma_start(
                            out=idxfbuf.rearrange("(f p) -> p f", p=16),
                            in_=sg_i[:])
                        nc.sync.dma_start(
                            out=cffbuf.rearrange("(f p) -> p f", p=16),
                            in_=sg_c[:])
                        # broadcast num_found to all partitions via rank-1 mm
                        nf_f = cp.tile([1, 1], f32)
                        nc.vector.tensor_copy(nf_f[:], nf[:])
                        nf_ps = cps.tile([P, 1], f32)
                        nc.tensor.matmul(nf_ps[:], lhsT=ones1f[:],
                                         rhs=nf_f[:], start=True, stop=True)
                        nf_bcast = cst.tile([P, 1], f32)
                        nc.vector.tensor_copy(nf_bcast[:], nf_ps[:])

                # ---- FFN ----------------------------------------------
                with (tc.tile_pool(name="idx", bufs=1) as ip,
                      tc.tile_pool(name="xtp", bufs=2) as xtp,
                      tc.tile_pool(name="wp", bufs=3) as wp,
                      tc.tile_pool(name="hp", bufs=1) as hp,
                      tc.tile_pool(name="yp", bufs=2) as yp,
                      tc.tile_pool(name="ps1", bufs=2, space="PSUM") as ps1,
                      tc.tile_pool(name="ps2", bufs=1, space="PSUM") as ps2):
                    if not dense:
                        NBC = CAP // P
                        idxf = ip.tile([P, NBC], f32)
                        nc.sync.dma_start(
                            out=idxf[:],
                            in_=idxfbuf.rearrange("(c p) -> p c", p=P))
                        cf_sb = ip.tile([P, NBC], f32)
                        nc.sync.dma_start(
                            out=cf_sb[:],
                            in_=cffbuf.rearrange("(c p) -> p c", p=P))
                        posj = ip.tile([P, NBC], f32)
                        nc.sync.dma_start(out=posj[:], in_=posj_in)
                        nf_bc = nf_bcast
                        inval = ip.tile([P, NBC], i32)
                        nc.vector.tensor_scalar(inval[:], posj[:],
                                                nf_bc[:, 0:1], None,
                                                op0=OP.is_ge)
                        sntf = ip.tile([P, NBC], f32)
                        nc.vector.memset(sntf[:], float(SENT))
                        idxe = ip.tile([P, NBC], f32)
                        nc.vector.select(idxe[:], inval[:], sntf[:], idxf[:])
                        idx_sb = ip.tile([P, NBC], i32)
                        nc.vector.tensor_copy(idx_sb[:], idxe[:])
                        gidx = ip.tile([P, NBC], i32)
                        nc.vector.tensor_scalar(gidx[:], idx_sb[:], Td - 1,
                                                0, op0=OP.min, op1=OP.max)
                        ident = ip.tile([P, P], f32)
                        make_identity(nc, ident[:])
                        if dbg:
                            nc.sync.dma_start(out=dbg_idx, in_=idx_sb[:])
                            nc.sync.dma_start(out=dbg_ccomp, in_=cf_sb[:])
                            nc.sync.dma_start(out=dbg_nf, in_=nf_bc[:])
                    for blk in range(NBLK):
                        ts0, tb = BLOCKS[blk]
                        mt_n = tb // P
                        xts = []
                        if dense:
                            for k in range(KD):
                                xt = xtp.tile([P, tb], wdt, tag=f"xt{k}",
                                              name=f"xt{k}")
                                nc.sync.dma_start(
                                    out=xt[:],
                                    in_=xT[k * P:(k + 1) * P, ts0:ts0 + tb])
                                xts.append(xt)
                        else:
                            for k in range(KD):
                                xt = xtp.tile([P, tb], wdt, tag=f"xt{k}",
                                              name=f"xt{k}")
                                xts.append(xt)
                            for j in range(mt_n):
                                c = ts0 // P + j
                                xg = xtp.tile([P, Dd], f32, tag="xg")
                                nc.gpsimd.indirect_dma_start(
                                    out=xg[:], out_offset=None,
                                    in_=xr,
                                    in_offset=IndirectOffsetOnAxis(
                                        ap=gidx[:, c:c + 1], axis=0))
                                for k in range(KD):
                                    pt = ps1.tile([P, P], f32, tag="ptr")
                                    nc.tensor.transpose(
                                        pt[:], xg[:, k * P:(k + 1) * P],
                                        ident[:])
                                    nc.vector.tensor_copy(
                                        xts[k][:, j * P:(j + 1) * P], pt[:])
                        hts = []
                        for m in range(MH):
                            w1m = wp.tile([P, KD * P], wdt, tag="w1m")
                            nc.sync.dma_start(out=w1m[:], in_=w1[m])
                            ph = ps1.tile([P, tb], f32, tag="ph")
                            for k in range(KD):
                                nc.tensor.matmul(
                                    ph[:], lhsT=w1m[:, k * P:(k + 1) * P],
                                    rhs=xts[k][:],
                                    start=(k == 0), stop=(k == KD - 1))
                            ht = hp.tile([P, tb], wdt, tag=f"ht{m}")
                            nc.scalar.activation(ht[:], ph[:], ACTF,
                                                 bias=b1s[:, m:m + 1],
                                                 scale=1.0)
                            hts.append(ht)
                        for d in range(ND):
                            pys = [ps2.tile([P, DCH], f32, tag=f"py{mt}",
                                            name=f"py{mt}")
                                   for mt in range(mt_n)]
                            for hk in range(MH):
                                for mt in range(mt_n):
                                    nc.tensor.matmul(
                                        pys[mt][:],
                                        lhsT=hts[hk][:, mt * P:(mt + 1) * P],
                                        rhs=w2all[:, hk * Dd + d * DCH:
                                                  hk * Dd + (d + 1) * DCH],
                                        start=(hk == 0), stop=False)
                            for mt in range(mt_n):
                                nc.tensor.matmul(
                                    pys[mt][:], lhsT=ones1r[:],
                                    rhs=b2s[:, d * DCH:(d + 1) * DCH],
                                    start=False, stop=True)
                            for mt in range(mt_n):
                                yq = yp.tile([P, DCH], ydt, tag=f"yq{mt}",
                                             name=f"yq{mt}")
                                c = ts0 // P + mt
                                if dense:
                                    nc.vector.tensor_scalar_mul(
                                        yq[:], pys[mt][:],
                                        ccol[:, c:c + 1])
                                    nc.sync.dma_start(
                                        out=ybuf[ts0 + mt * P:
                                                 ts0 + (mt + 1) * P,
                                                 d * DCH:(d + 1) * DCH],
                                        in_=yq[:])
                                else:
                                    nc.vector.tensor_scalar_mul(
                                        yq[:], pys[mt][:],
                                        cf_sb[:, c:c + 1])
                                    nc.gpsimd.indirect_dma_start(
                                        out=ybuf[:],
                                        out_offset=IndirectOffsetOnAxis(
                                            ap=idx_sb[:, c:c + 1], axis=0),
                                        in_=yq[:], in_offset=None,
                                        element_offset=d * DCH,
                                        bounds_check=Td - 1,
                                        oob_is_err=False)

            nc.gpsimd.collective_compute(
                "ReduceScatter", OP.add, replica_groups=RG,
                ins=[ybuf.opt()], outs=[yshard.opt()])
            nc.sync.dma_start(out=out, in_=yshard[:])

    nc.compile()
    meta = dict(D=Dd, H=Hd, T=Td, E=Ed, TPC=TPC, CAP=CAP)
    return nc, meta


# ----------------------------------------------------------------------------
def make_in_maps(inputs, dims=None, dense=False, wdtype="f32r"):
    """Shard full inputs into per-core input maps (host-side, numpy only)."""
    d_ = dims or {}
    Td = d_.get("T", T)
    Dd = d_.get("D", D)
    TPC = Td // NC
    x = np.asarray(inputs["x"], dtype=np.float32)
    x2 = np.ascontiguousarray(x.reshape(Td, Dd))
    temp = np.float32(inputs["temperature"])
    gws = np.ascontiguousarray(np.asarray(inputs["gate_w"], np.float32) / temp)
    gbs = np.ascontiguousarray(np.asarray(inputs["gate_b"], np.float32) / temp)
    W1 = np.asarray(inputs["W1"], np.float32)
    b1_ = np.asarray(inputs["b1"], np.float32)
    W2 = np.asarray(inputs["W2"], np.float32)
    b2_ = np.asarray(inputs["b2"], np.float32)
    if wdtype == "bf16":
        import ml_dtypes
        wnp = ml_dtypes.bfloat16
    else:
        wnp = np.float32
    W1 = W1.astype(wnp)
    W2 = W2.astype(wnp)
    b2_ = b2_.astype(wnp)
    # retile W1 per expert: [D, H] -> [MH, P, KD*P] with
    # w1t[m, p, k*128+h] = W1[k*128+p, m*128+h]
    Hd = W1.shape[2]
    KDn, MHn = Dd // 128, Hd // 128
    W1 = np.ascontiguousarray(
        W1.reshape(-1, KDn, 128, MHn, 128).transpose(0, 3, 2, 1, 4)
        .reshape(-1, MHn, 128, KDn * 128))
    if dense:
        xT_np = np.ascontiguousarray(x2.T).astype(wnp)
    else:
        CAP = d_.get("NCAP", NCAP)
        F16 = Td // 16
        iota16_np = ((np.arange(16)[:, None] + 16 * np.arange(F16)[None, :])
                     .astype(np.float32) + 1.0)
        posj_np = (np.arange(CAP // P)[None, :] * P
                   + np.arange(P)[:, None]).astype(np.float32)
    in_maps = []
    for rk in range(NC):
        m = {
            "xsT": np.ascontiguousarray(x2[rk * TPC:(rk + 1) * TPC].T),
            "w1": np.ascontiguousarray(W1[rk]),
            "b1": np.ascontiguousarray(b1_[rk]),
            "w2": np.ascontiguousarray(W2[rk]),
            "b2": np.ascontiguousarray(b2_[rk]),
            "gw": gws,
            "gb": gbs,
        }
        if dense:
            m["xT"] = xT_np
        else:
            m["xr"] = x2
            m["iota16"] = iota16_np
            m["posj"] = posj_np
        in_maps.append(m)
    return in_maps


_BUILT = {}


def run_hw(inputs, dims=None, trace=False, act="gelu", dense=False,
           wdtype="f32r", ybf16=False):
    """Run on hardware via run_bass_kernel_spmd; returns (out_full, results)."""
    from concourse.bass_utils import run_bass_kernel_spmd
    key = (dense, act, wdtype, ybf16, tuple(sorted((dims or {}).items())))
    if key not in _BUILT:
        _BUILT[key] = build_moe(dims=dims, dense=dense, act=act,
                                wdtype=wdtype, ybf16=ybf16)
    nc, meta = _BUILT[key]
    in_maps = make_in_maps(inputs, dims=dims, dense=dense, wdtype=wdtype)
    res = run_bass_kernel_spmd(nc, in_maps, list(range(NC)), trace=trace)
    shards = [np.asarray(res.results[i]["out"], dtype=np.float32)
              for i in range(NC)]
    out_full = np.concatenate(shards, axis=0)
    if not dims:
        out_full = out_full.reshape(B, S, D)
    return out_full, res


def kernel(**inputs):
    out, _ = run_hw(inputs, dims=None, trace=False, dense=False)
    return out
